# revision 15
# baseline (speedup 1.0000x reference)
"""Trainium2 Bass kernel: causal sliding-window GQA self-attention.

Problem: B=2, T=2048, C=2048, 16 q-heads / 4 kv-heads, head_dim=128,
RoPE, sliding window 512, projections Wq/Wk/Wv/Wo.

Sharding: 8 cores = DP(batch=2) x TP(head-groups=4).  Core c handles
batch c//4 and q-heads [4*(c%4), 4*(c%4)+4) (one kv head c%4).  Each
core computes a partial output contribution [T, C]; the host sums the
4 head-group partials per batch (and divides by the 64^2 weight
pre-scale, see below).

Per-core kernel:
  - Projections and the Wo matmul run in fp8-e4m3 DoubleRow perf mode
    (two contraction rows per PE pass, 0.5 cycles/output-column: 4x the
    bf16 MAC rate).  Full precision is recovered with a 3-term hi-lo
    split: x ~ x_hi + x_lo (fp8 pair, host-prepared), W ~ W_hi + W_lo,
    and x@W ~ xhi@Whi + xlo@Whi + xhi@Wlo -- 24 DoubleRow matmuls per
    2048-contraction tile vs 16 bf16 matmuls, i.e. 0.75x the PE time
    with accuracy slightly better than a bf16 matmul.  Weights are
    pre-scaled by 64 on the host so W values (~0.02) sit in the fp8
    normal range; the 1/64 is folded into the RoPE eviction tables for
    Q/K, carried harmlessly through V->y->Wo for the rest, and divided
    out on the host (out = psum / 4096).
  - Attention stays bf16 (contraction is only 128 there, DoubleRow
    buys nothing at equal accuracy) but batches the 4 GQA q-heads of
    the shared kv-head into one free-dim-512 stream: per (128-query
    block, 128-key block) ONE score matmul [keys, 4*128] and ONE PV
    accumulation into ynT [hd, 4*128], quartering instruction counts.
  - exp on ScalarE (no max subtraction; max |score| ~5.5 on this input
    distribution), band-mask multiplies only on the two edge key
    blocks (leading edge on GpSimd, diagonal on DVE), softmax
    denominators via DVE pairwise adds + one GpSimd
    partition_all_reduce, reciprocal on DVE.
  - ynT is written as an fp8 hi/lo pair (DVE mul + ScalarE cast + DVE
    sub) feeding the 3-term DoubleRow Wo.
  - PE program order per query block: scores(qb) -> Wo(qb-1) -> PV(qb),
    so the late-emitted Wo matmuls fill the exp/mask latency window of
    the current block's attention chain.
  - x streams tb-major (512 t-columns at a time across all 16
    contraction row-blocks) so the first projection tile is gated on
    ~2.5MB of DMA instead of the whole 8MB x load.
"""

import os
import sys

for _p in ("/opt/trn_rl_repo", "/root/.axon_site/_ro/trn_rl_repo"):
    if os.path.isdir(_p) and _p not in sys.path:
        sys.path.append(_p)

import numpy as np
import ml_dtypes

BF16 = ml_dtypes.bfloat16
F8 = ml_dtypes.float8_e4m3fn

B, T, C = 2, 2048, 2048
H, KVH, HD = 16, 4, 128
WIN = 512
ROPE_BASE = 10000.0
NCORES = 8
TPG = 4           # tensor-parallel group count (head groups)
HPG = H // TPG    # q-heads per core
SCALE = 1.0 / float(np.sqrt(np.float32(HD)))
NWINB = WIN // 128 + 1   # 5 key blocks cover the 640-wide window
NCB = C // 128           # contraction row-blocks for projections
WSC = 64.0               # host weight pre-scale (fp8 subnormal dodge)

_NC_CACHE = {}


def _rope_tables(t_len):
    # Match reference: angles computed in float32.
    inv = (1.0 / (np.float32(ROPE_BASE) ** (np.arange(0, HD, 2, dtype=np.float32) / np.float32(HD)))).astype(np.float32)
    ang = np.arange(t_len, dtype=np.float32)[None, :] * inv[:, None]   # [64, T]
    cosT = np.concatenate([np.cos(ang), np.cos(ang)], axis=0)          # [128, T]
    sinT = np.sin(ang)
    sin_swap = np.concatenate([-sinT, sinT], axis=0)                   # [128, T]
    return cosT.astype(np.float32), sin_swap.astype(np.float32)


def _band_maskT4():
    # maskT[c, r] = 1 iff query row r may attend key col c of the
    # 640-wide window (c = j - (qs - 512)):  r+1 <= c <= r+512.
    # Replicated x4 along the free dim for the 4-head-batched layout.
    r = np.arange(128)[None, :]
    c = np.arange(NWINB * 128)[:, None]
    m = ((r + 1 <= c) & (c <= r + WIN)).astype(np.float32)             # [640, 128]
    return np.tile(m, (1, HPG))                                        # [640, 512]


def _split_f8(a):
    """fp8 hi/lo pair: a ~ hi + lo with ~7-bit effective mantissa."""
    a32 = np.asarray(a, np.float32)
    hi = a32.astype(F8)
    lo = (a32 - hi.astype(np.float32)).astype(F8)
    return hi, lo


def build_nc(t_len=T):
    """Build + compile the per-core Bass module (SPMD, identical on all cores)."""
    import concourse.mybir as mybir
    import concourse.tile as tile
    from concourse import bacc
    from concourse import bass_isa

    dt = mybir.dt
    DRow = mybir.MatmulPerfMode.DoubleRow
    NQB = t_len // 128        # query/key blocks
    NTB = t_len // 512        # 512-wide t-blocks for projections

    nc = bacc.Bacc("TRN2", target_bir_lowering=False, debug=False, num_devices=NCORES)

    def din(name, shape, d=dt.float8e4):
        return nc.dram_tensor(name, shape, d, kind="ExternalInput").ap()

    xhi_d = din("xhi", [C, t_len])
    xlo_d = din("xlo", [C, t_len])
    wqhi_d = din("wqhi", [C, HPG * HD])
    wqlo_d = din("wqlo", [C, HPG * HD])
    wkhi_d = din("wkhi", [C, HD])
    wklo_d = din("wklo", [C, HD])
    wvhi_d = din("wvhi", [C, HD])
    wvlo_d = din("wvlo", [C, HD])
    wohi_d = din("wohi", [HPG * HD, C])
    wolo_d = din("wolo", [HPG * HD, C])
    cosq_d = din("cosq", [HD, t_len], dt.bfloat16)
    sinq_d = din("sinq", [HD, t_len], dt.bfloat16)
    cosk_d = din("cosk", [HD, t_len], dt.bfloat16)
    sink_d = din("sink", [HD, t_len], dt.bfloat16)
    maskT4_d = din("maskT4", [NWINB * 128, HPG * 128], dt.bfloat16)
    out_d = nc.dram_tensor("out", [t_len, C], dt.bfloat16, kind="ExternalOutput").ap()

    with tile.TileContext(nc) as tc:
        with tc.tile_pool(name="persist", bufs=1) as pp:
            f8 = dt.float8e4
            xhi_sb = pp.tile([128, NCB * t_len], f8, tag="xhi")
            xlo_sb = pp.tile([128, NCB * t_len], f8, tag="xlo")
            wqhi_sb = pp.tile([128, NCB * HPG * HD], f8, tag="wqhi")
            wqlo_sb = pp.tile([128, NCB * HPG * HD], f8, tag="wqlo")
            wkhi_sb = pp.tile([128, NCB * HD], f8, tag="wkhi")
            wklo_sb = pp.tile([128, NCB * HD], f8, tag="wklo")
            wvhi_sb = pp.tile([128, NCB * HD], f8, tag="wvhi")
            wvlo_sb = pp.tile([128, NCB * HD], f8, tag="wvlo")
            wohi_sb = pp.tile([128, HPG * C], f8, tag="wohi")
            wolo_sb = pp.tile([128, HPG * C], f8, tag="wolo")
            cosq_sb = pp.tile([128, t_len], dt.bfloat16, tag="cosq")
            sinq_sb = pp.tile([128, t_len], dt.bfloat16, tag="sinq")
            cosk_sb = pp.tile([128, t_len], dt.bfloat16, tag="cosk")
            sink_sb = pp.tile([128, t_len], dt.bfloat16, tag="sink")
            maskT4_sb = pp.tile([128, NWINB * HPG * 128], dt.bfloat16, tag="maskT4")
            QT4_sb = pp.tile([128, NQB * HPG * 128], dt.bfloat16, tag="QT4")
            KT_sb = pp.tile([128, t_len], dt.bfloat16, tag="KT")
            V_sb = pp.tile([128, t_len], dt.bfloat16, tag="V")

            # 3D chunk views for DoubleRow operand pairing.
            def xv(ts):
                return ts[:].rearrange("p (c t) -> p c t", t=t_len)

            def wv_(ts, m):
                return ts[:].rearrange("p (c m) -> p c m", m=m)

            # ---- DMA emission order: first consumers first; x streams
            # tb-major so the first projection tile gates on ~2.5MB.
            def xload(xs, xd, tsl):
                for cb in range(0, NCB, 4):
                    nc.sync.dma_start(
                        xv(xs)[:, cb:cb + 4, tsl],
                        xd[cb * 128:(cb + 4) * 128, tsl].rearrange("(c p) t -> p c t", p=128))

            # First projection tile gates on: wv_hi + x_hi(tb0) + wv_lo +
            # x_lo(tb0) -- emit exactly in that order so the PE's first hi*hi
            # matmuls start as early as possible.
            nc.sync.dma_start(wv_(wvhi_sb, HD), wvhi_d.rearrange("(c p) h -> p c h", p=128))
            for tb in range(NTB):
                tsl = slice(tb * 512, (tb + 1) * 512)
                if tb == 0:
                    xload(xhi_sb, xhi_d, tsl)
                    nc.sync.dma_start(wv_(wvlo_sb, HD), wvlo_d.rearrange("(c p) h -> p c h", p=128))
                    xload(xlo_sb, xlo_d, tsl)
                    nc.sync.dma_start(wv_(wkhi_sb, HD), wkhi_d.rearrange("(c p) h -> p c h", p=128))
                    nc.sync.dma_start(wv_(wklo_sb, HD), wklo_d.rearrange("(c p) h -> p c h", p=128))
                else:
                    xload(xhi_sb, xhi_d, tsl)
                    xload(xlo_sb, xlo_d, tsl)
                if tb == 0:
                    nc.sync.dma_start(wv_(wqhi_sb, HPG * HD), wqhi_d.rearrange("(c p) m -> p c m", p=128))
                    nc.sync.dma_start(wv_(wqlo_sb, HPG * HD), wqlo_d.rearrange("(c p) m -> p c m", p=128))
                    nc.sync.dma_start(cosk_sb[:], cosk_d)
                    nc.sync.dma_start(sink_sb[:], sink_d)
                    nc.sync.dma_start(cosq_sb[:], cosq_d)
                    nc.sync.dma_start(sinq_sb[:], sinq_d)
                if tb == 1:
                    nc.sync.dma_start(maskT4_sb[:].rearrange("p (m c) -> p m c", c=HPG * 128),
                                      maskT4_d.rearrange("(m p) c -> p m c", p=128))
                    nc.sync.dma_start(wv_(wohi_sb, C), wohi_d.rearrange("(h p) c -> p h c", p=128))
                    nc.sync.dma_start(wv_(wolo_sb, C), wolo_d.rearrange("(h p) c -> p h c", p=128))

            TERMS_V = ((xhi_sb, wvhi_sb), (xlo_sb, wvhi_sb), (xhi_sb, wvlo_sb))
            TERMS_K = ((xhi_sb, wkhi_sb), (xlo_sb, wkhi_sb), (xhi_sb, wklo_sb))
            TERMS_Q = ((xhi_sb, wqhi_sb), (xlo_sb, wqhi_sb), (xhi_sb, wqlo_sb))

            # ---------------- projections ----------------
            with tc.tile_pool(name="proj_ps", bufs=5, space="PSUM") as pps, \
                 tc.tile_pool(name="v_ps", bufs=2, space="PSUM") as vpp, \
                 tc.tile_pool(name="rope_scr", bufs=3) as rsc:

                def rope_evict(ps, dst, cos_sb, sin_sb, tb, dst3=None):
                    sl = slice(tb * 512, (tb + 1) * 512)
                    t1 = rsc.tile([128, 512], dt.float32, tag="t1")
                    t2 = rsc.tile([128, 512], dt.float32, tag="t2")
                    nc.vector.tensor_mul(t1[:], ps[:], cos_sb[:, sl])
                    nc.vector.tensor_mul(t2[0:64, :], ps[64:128, :], sin_sb[0:64, sl])
                    nc.vector.tensor_mul(t2[64:128, :], ps[0:64, :], sin_sb[64:128, sl])
                    if dst3 is None:
                        nc.gpsimd.tensor_add(dst, t1[:], t2[:])
                    else:
                        r3 = lambda a: a.rearrange("p (a b) -> p a b", b=128)
                        nc.gpsimd.tensor_add(dst3, r3(t1[:]), r3(t2[:]))

                for tb in range(NTB):
                    tsl = slice(tb * 512, (tb + 1) * 512)
                    # V tiles, direct [t, hd] layout (no transpose needed)
                    for j in range(4):
                        t0 = (tb * 4 + j) * 128
                        vps = vpp.tile([128, 128], dt.float32, tag="vps", name="vps")
                        k = 0
                        for xs, ws in TERMS_V:
                            for cb in range(0, NCB, 2):
                                nc.tensor.matmul(
                                    vps[:], xv(xs)[:, cb:cb + 2, t0:t0 + 128],
                                    wv_(ws, HD)[:, cb:cb + 2, :],
                                    start=(k == 0), stop=(k == 23), perf_mode=DRow)
                                k += 1
                        nc.any.tensor_copy(V_sb[:, t0:t0 + 128], vps[:])
                    # K tile
                    kps = pps.tile([128, 512], dt.float32, tag="ps", name="kps")
                    k = 0
                    for xs, ws in TERMS_K:
                        for cb in range(0, NCB, 2):
                            nc.tensor.matmul(
                                kps[:], wv_(ws, HD)[:, cb:cb + 2, :],
                                xv(xs)[:, cb:cb + 2, tsl],
                                start=(k == 0), stop=(k == 23), perf_mode=DRow)
                            k += 1
                    rope_evict(kps, KT_sb[:, tsl], cosk_sb, sink_sb, tb)
                    # Q tiles (4 heads), evicted into the per-qb 4-head layout
                    for h in range(HPG):
                        qps = pps.tile([128, 512], dt.float32, tag="ps", name="qps")
                        k = 0
                        for xs, ws in TERMS_Q:
                            for cb in range(0, NCB, 2):
                                nc.tensor.matmul(
                                    qps[:], wv_(ws, HPG * HD)[:, cb:cb + 2, h * HD:(h + 1) * HD],
                                    xv(xs)[:, cb:cb + 2, tsl],
                                    start=(k == 0), stop=(k == 23), perf_mode=DRow)
                                k += 1
                        dst3 = QT4_sb[:].rearrange("p (q s) -> p q s", s=HPG * 128)[
                            :, 4 * tb:4 * tb + 4, h * 128:(h + 1) * 128]
                        rope_evict(qps, None, cosq_sb, sinq_sb, tb, dst3=dst3)

            # ---------------- attention + Wo ----------------
            with tc.tile_pool(name="st_ps", bufs=3, space="PSUM") as stp, \
                 tc.tile_pool(name="acc_ps", bufs=2, space="PSUM") as accp, \
                 tc.tile_pool(name="wo_ps", bufs=3, space="PSUM") as wop, \
                 tc.tile_pool(name="attn_sb", bufs=6) as asb, \
                 tc.tile_pool(name="den_sb", bufs=2) as dsb, \
                 tc.tile_pool(name="yn_sb", bufs=3) as ysb, \
                 tc.tile_pool(name="out_sb", bufs=2) as osb:
                Exp = mybir.ActivationFunctionType.Exp
                Copy = mybir.ActivationFunctionType.Copy

                def wo_mms(yhi, ylo, cb4s):
                    """Wo DoubleRow matmuls for the given output-column chunks."""
                    y3 = lambda t: t[:].rearrange("p (k m) -> p k m", m=128)
                    tiles = []
                    for cb4 in cb4s:
                        csl = slice(cb4 * 512, (cb4 + 1) * 512)
                        wps = wop.tile([128, 512], dt.float32, tag="wps", name="wps")
                        k = 0
                        for ys, ws in ((yhi, wohi_sb), (ylo, wohi_sb), (yhi, wolo_sb)):
                            for p2 in range(2):
                                nc.tensor.matmul(
                                    wps[:], y3(ys)[:, 2 * p2:2 * p2 + 2, :],
                                    wv_(ws, C)[:, 2 * p2:2 * p2 + 2, csl],
                                    start=(k == 0), stop=(k == 5), perf_mode=DRow)
                                k += 1
                        tiles.append((cb4, wps))
                    return tiles

                def wo_evict(wo_qb, ostg, tiles, dve):
                    # evictions: one rides DVE (early, frees the bank the 4th
                    # chunk's matmuls rotate onto), the rest ride Pool behind
                    # this block's mask/allreduce.
                    for cb4, wps in tiles:
                        csl = slice(cb4 * 512, (cb4 + 1) * 512)
                        if dve:
                            nc.vector.tensor_copy(ostg[:, csl], wps[:])
                        else:
                            nc.gpsimd.tensor_copy(ostg[:, csl], wps[:])
                        if wo_qb >= NQB - 2:
                            nc.sync.dma_start(out_d[wo_qb * 128:(wo_qb + 1) * 128, csl],
                                              ostg[:, csl])

                pend = []
                for qb in range(NQB):
                    nwin = min(qb, NWINB - 1) + 1
                    qsl = slice(qb * 512, (qb + 1) * 512)
                    # scores for all 4 heads at once, one matmul per key block,
                    # exp chasing each score so PSUM banks recycle fast
                    pms = []
                    for i in range(nwin):
                        jb = qb - nwin + 1 + i
                        m = i + NWINB - nwin
                        st = stp.tile([128, 512], dt.float32, tag="st", name="st")
                        nc.tensor.matmul(st[:], KT_sb[:, jb * 128:(jb + 1) * 128],
                                         QT4_sb[:, qsl], start=True, stop=True)
                        pexp = asb.tile([128, 512], dt.bfloat16, tag="pexp", name="pexp")
                        nc.scalar.activation(pexp[:], st[:], Exp)
                        if m == 0:
                            pm = asb.tile([128, 512], dt.bfloat16, tag="pmask")
                            nc.gpsimd.tensor_mul(pm[:], pexp[:], maskT4_sb[:, 0:512])
                            pms.append(pm)
                        elif m == NWINB - 1:
                            pm = asb.tile([128, 512], dt.bfloat16, tag="pmask")
                            nc.vector.tensor_mul(pm[:], pexp[:],
                                                 maskT4_sb[:, (NWINB - 1) * 512:NWINB * 512])
                            pms.append(pm)
                        else:
                            pms.append(pexp)
                    # Wo of the previous block (3 of 4 chunks): dense PE work
                    # filling the exp/mask latency window of this block's chain.
                    if pend:
                        wo_qb, yhi_p, ylo_p, ostg_p = pend.pop(0)
                        wo_tiles = wo_mms(yhi_p, ylo_p, (0, 1, 2))
                    else:
                        wo_qb = None
                    # PV accumulation (4 heads batched)
                    acc = accp.tile([128, 512], dt.float32, tag="acc", name="acc")
                    for i in range(nwin):
                        jb = qb - nwin + 1 + i
                        nc.tensor.matmul(acc[:], V_sb[:, jb * 128:(jb + 1) * 128], pms[i][:],
                                         start=(i == 0), stop=(i == nwin - 1))
                    # softmax denominator: tree adds (DVE) + partition reduce
                    work = [p[:] for p in pms]
                    while len(work) > 1:
                        nxt = []
                        for a, b in zip(work[0::2], work[1::2]):
                            t = asb.tile([128, 512], dt.bfloat16, tag="padd", name="padd")
                            nc.vector.tensor_add(t[:], a, b)
                            nxt.append(t[:])
                        if len(work) % 2:
                            nxt.append(work[-1])
                        work = nxt
                    sbc = dsb.tile([128, 512], dt.float32, tag="sbc")
                    nc.gpsimd.partition_all_reduce(sbc[:], work[0], channels=128,
                                                   reduce_op=bass_isa.ReduceOp.add)
                    rbc = dsb.tile([128, 512], dt.bfloat16, tag="rbc")
                    with nc.allow_low_precision("softmax denominator reciprocal; 2e-2 rel-err budget"):
                        nc.vector.reciprocal(rbc[:], sbc[:])
                    # previous block's Wo: first chunk evicts early on DVE
                    # (freeing the bank the 4th chunk rotates onto), the 4th
                    # chunk's matmuls fill the denominator window, remaining
                    # evictions ride Pool behind mask0/allreduce.
                    if wo_qb is not None:
                        wo_evict(wo_qb, ostg_p, wo_tiles[:1], dve=True)
                        wo_tiles += wo_mms(yhi_p, ylo_p, (3,))
                    # ynT = acc * rbc, written as an fp8 hi/lo pair for Wo
                    t32 = ysb.tile([128, 512], dt.float32, tag="t32", name="t32")
                    nc.vector.tensor_mul(t32[:], acc[:], rbc[:])
                    yhi = ysb.tile([128, 512], f8, tag="yhi", name="yhi")
                    nc.vector.tensor_copy(yhi[:], t32[:])
                    ylo = ysb.tile([128, 512], f8, tag="ylo", name="ylo")
                    with nc.allow_low_precision("fp8 lo residual of ynT pair"):
                        nc.vector.tensor_sub(ylo[:], t32[:], yhi[:])
                    if wo_qb is not None:
                        wo_evict(wo_qb, ostg_p, wo_tiles[1:], dve=False)
                        if wo_qb < NQB - 2:
                            nc.sync.dma_start(out_d[wo_qb * 128:(wo_qb + 1) * 128, :], ostg_p[:])
                    ostg = osb.tile([128, C], dt.bfloat16, tag="ostg", name="ostg")
                    pend.append((qb, yhi, ylo, ostg))
                while pend:
                    wo_qb, yhi_p, ylo_p, ostg_p = pend.pop(0)
                    tiles = wo_mms(yhi_p, ylo_p, (0, 1, 2, 3))
                    wo_evict(wo_qb, ostg_p, tiles[:1], dve=True)
                    wo_evict(wo_qb, ostg_p, tiles[1:], dve=False)
                    if wo_qb < NQB - 2:
                        nc.sync.dma_start(out_d[wo_qb * 128:(wo_qb + 1) * 128, :], ostg_p[:])

    nc.compile()
    return nc


def _get_nc(t_len=T):
    if t_len not in _NC_CACHE:
        _NC_CACHE[t_len] = build_nc(t_len)
    return _NC_CACHE[t_len]


def host_inputs(x, Wq, Wk, Wv, Wo, t_len=T):
    """Per-core input shards (8 dicts)."""
    x = np.asarray(x, np.float32)
    Wq = np.asarray(Wq, np.float32) * WSC
    Wk = np.asarray(Wk, np.float32) * WSC
    Wv = np.asarray(Wv, np.float32) * WSC
    Wo = np.asarray(Wo, np.float32) * WSC
    cosT, sin_swap = _rope_tables(t_len)
    common = {
        "cosq": (cosT * (SCALE / WSC)).astype(BF16),
        "sinq": (sin_swap * (SCALE / WSC)).astype(BF16),
        "cosk": (cosT / WSC).astype(BF16),
        "sink": (sin_swap / WSC).astype(BF16),
        "maskT4": _band_maskT4().astype(BF16),
    }
    in_maps = []
    for core in range(NCORES):
        b, hg = core // TPG, core % TPG
        m = dict(common)
        m["xhi"], m["xlo"] = _split_f8(np.ascontiguousarray(x[b, :t_len, :].T))
        m["wqhi"], m["wqlo"] = _split_f8(Wq[:, hg * HPG * HD:(hg + 1) * HPG * HD])
        m["wkhi"], m["wklo"] = _split_f8(Wk[:, hg * HD:(hg + 1) * HD])
        m["wvhi"], m["wvlo"] = _split_f8(Wv[:, hg * HD:(hg + 1) * HD])
        m["wohi"], m["wolo"] = _split_f8(Wo[hg * HPG * HD:(hg + 1) * HPG * HD, :])
        in_maps.append(m)
    return in_maps


def kernel(x, Wq, Wk, Wv, Wo):
    from concourse import bass_utils

    nc = _get_nc(T)
    in_maps = host_inputs(x, Wq, Wk, Wv, Wo, T)
    res = bass_utils.run_bass_kernel_spmd(nc, in_maps, core_ids=list(range(NCORES)))
    out = np.zeros((B, T, C), np.float32)
    for core in range(NCORES):
        out[core // TPG] += res.results[core]["out"].astype(np.float32)
    out *= 1.0 / (WSC * WSC)
    return out


def core_reference(x_b, Wq, Wk, Wv, Wo, hg, t_len=T):
    """Numpy reference of one core's partial output (f32 math, for dev tests)."""
    xb = np.asarray(x_b, np.float64)[:t_len]
    q = xb @ np.float64(Wq[:, hg * HPG * HD:(hg + 1) * HPG * HD])    # [T, 512]
    k = xb @ np.float64(Wk[:, hg * HD:(hg + 1) * HD])                # [T, 128]
    v = xb @ np.float64(Wv[:, hg * HD:(hg + 1) * HD])
    cosT, sin_swap = _rope_tables(t_len)
    cos = cosT.T.astype(np.float64)
    sinsw = sin_swap.T.astype(np.float64)

    def rope(z):
        zsw = np.concatenate([z[:, HD // 2:], z[:, :HD // 2]], axis=1)
        sgn = np.concatenate([sinsw[:, :HD // 2], sinsw[:, HD // 2:]], axis=1)
        return z * cos + zsw * sgn

    out = np.zeros((t_len, C), np.float64)
    i = np.arange(t_len)[:, None]
    j = np.arange(t_len)[None, :]
    allowed = (j <= i) & (i - j < WIN)
    kr = rope(k)
    for h in range(HPG):
        qh = rope(q[:, h * HD:(h + 1) * HD]) * SCALE
        s = qh @ kr.T
        s = np.where(allowed, s, -np.inf)
        p = np.exp(s - s.max(axis=1, keepdims=True))
        p /= p.sum(axis=1, keepdims=True)
        y = p @ v
        out += y @ np.float64(Wo[hg * HPG * HD + h * HD: hg * HPG * HD + (h + 1) * HD, :])
    return out.astype(np.float32)
